# revision 14
# baseline (speedup 1.0000x reference)
"""CIN block kernel for Trainium2 (8 NeuronCores, data-parallel over batch).

Reference computation (per layer l, h0 = feat):
    out_l[b,k,d] = relu( sum_{i,j} W_l[k,i,j] * h_l[b,i,d] * feat[b,j,d] + b_l[k] )
    h_{l+1} = out_l[:, :K/2, :]   (split-half, except last layer)
    result  = concat([out0[:,128:], out1[:,128:], out2[:,:]], axis=1).sum(-1)

Mapping (per core, B_local=64, BD = B_local*D = 2048):
    Tensors live as [channel, (b,d)] with (b,d) flattened on the free dim.
    z_j[i, bd] = h[i, bd] * feat[j, bd]: feat row j is replicated across the
    128 partitions by a broadcast DMA from DRAM (src partition-stride 0) into
    a persistent fb buffer, reused by layers 1+2; z_j is one fp16 DVE
    multiply. out[k, bd] = sum_j Wt_j[i,k].T @ z_j: PE matmuls accumulating
    in PSUM (fp32), drained by the scalar engine as relu(x + b).
    Layer 0 (h = feat) contracts 1024 (i,j) pairs in 8 chunks of 128; both
    replicated factors (featH, featR) are host-prepared inputs, so layer 0 is
    one DVE multiply + matmuls per chunk.
    The batch is processed in two halves of 1024 positions so the 32 fb
    tiles (8 MB fp16) fit in SBUF. Everything is fp16 with fp32 accumulation.
"""

import os
import sys

import numpy as np

for _p in ("/opt/trn_rl_repo", "/root/.axon_site/_ro/trn_rl_repo"):
    if os.path.isdir(_p) and _p not in sys.path:
        sys.path.insert(0, _p)

import concourse.bacc as bacc
import concourse.bass as bass
import concourse.mybir as mybir
import concourse.tile as tile
from concourse.bass_utils import run_bass_kernel_spmd

F32 = mybir.dt.float32
F16 = mybir.dt.float16
RELU = mybir.ActivationFunctionType.Relu
AXX = mybir.AxisListType.X

NCORES = 8
B, F0, D = 512, 32, 32
BL = B // NCORES          # 64 batch rows per core
BD = BL * D               # 2048 free positions per core
NT = 512                  # free-dim tile (one PSUM bank)
HB = 1024                 # half of BD
K = 256                   # channels per layer
H = 128                   # hidden rows fed to layers 1,2 (split-half of 256)
NJ0 = F0 * F0 // 128      # 8 partition-chunks for layer-0 (i,j) pairs

_CACHE = {}
LAST_RESULTS = None


def _build_program():
    nc = bacc.Bacc("TRN2", target_bir_lowering=False, debug=False)

    featT_d = nc.dram_tensor("featT16", [F0, BD], F16, kind="ExternalInput").ap()
    featR_d = nc.dram_tensor("featR", [128, BD], F16, kind="ExternalInput").ap()
    featH_d = nc.dram_tensor("featH", [128, NJ0 * BD], F16, kind="ExternalInput").ap()
    w0_d = nc.dram_tensor("w0t", [128, NJ0 * K], F16, kind="ExternalInput").ap()
    w1_d = nc.dram_tensor("w1t", [128, F0 * K], F16, kind="ExternalInput").ap()
    w2_d = nc.dram_tensor("w2t", [128, F0 * K], F16, kind="ExternalInput").ap()
    b0_d = nc.dram_tensor("b0t", [128, 2], F32, kind="ExternalInput").ap()
    b1_d = nc.dram_tensor("b1t", [128, 2], F32, kind="ExternalInput").ap()
    b2_d = nc.dram_tensor("b2t", [128, 2], F32, kind="ExternalInput").ap()
    out_d = nc.dram_tensor("out", [512, BL], F32, kind="ExternalOutput").ap()

    with tile.TileContext(nc) as tc:
        with (
            tc.tile_pool(name="const", bufs=1) as const,
            tc.tile_pool(name="ps", bufs=8, space="PSUM") as ps,
            tc.tile_pool(name="zp16", bufs=4) as zp16,
            tc.tile_pool(name="zp0", bufs=4) as zp0,
            tc.tile_pool(name="dp", bufs=3) as dp,
        ):
            featR = const.tile([128, BD], F16)
            nc.sync.dma_start(featR, featR_d)
            featH = const.tile([128, NJ0 * BD], F16)
            nc.sync.dma_start(featH, featH_d)
            w0 = const.tile([128, NJ0 * K], F16)
            nc.sync.dma_start(w0, w0_d)
            w1 = const.tile([128, F0 * K], F16)
            nc.sync.dma_start(w1, w1_d)
            w2 = const.tile([128, F0 * K], F16)
            nc.sync.dma_start(w2, w2_d)
            b0 = const.tile([128, 2], F32)
            nc.sync.dma_start(b0, b0_d)
            b1 = const.tile([128, 2], F32)
            nc.sync.dma_start(b1, b1_d)
            b2 = const.tile([128, 2], F32)
            nc.sync.dma_start(b2, b2_d)

            h1 = const.tile([128, BD], F16)
            h2 = const.tile([128, BD], F16)
            # 32 broadcast tiles for one half, split in two for cross-half overlap
            fbh = [const.tile([128, 16 * HB], F16, name=f"fbh{i}") for i in range(2)]
            r0 = const.tile([128, BL], F32)
            r1 = const.tile([128, BL], F32)
            r2a = const.tile([128, BL], F32)
            r2b = const.tile([128, BL], F32)

            def drain(o_ps, bias_ap, t, h_out, r_out):
                """relu(psum + bias) -> fp16 h slice, or f32 tile + d-reduce."""
                if h_out is not None:
                    nc.scalar.activation(
                        h_out[:, t * NT : (t + 1) * NT], o_ps, RELU, bias=bias_ap
                    )
                else:
                    dx = dp.tile([128, NT], F32, tag="d", name=f"d_{t}")
                    nc.scalar.activation(dx, o_ps, RELU, bias=bias_ap)
                    nc.vector.reduce_sum(
                        r_out[:, t * (NT // D) : (t + 1) * (NT // D)],
                        dx.rearrange("p (b d) -> p b d", d=D),
                        axis=AXX,
                    )

            for half in range(2):
                hoff = half * HB

                # fb prefetch: feat row j broadcast to 128 partitions via DMA
                for j in range(F0):
                    eng = nc.sync if j % 2 == 0 else nc.gpsimd
                    eng.dma_start(
                        fbh[j // 16][:, (j % 16) * HB : (j % 16 + 1) * HB],
                        featT_d[j : j + 1, hoff : hoff + HB].to_broadcast([128, HB]),
                    )

                # ---------------- Layer 0 (h = feat) ----------------
                for t in (2 * half, 2 * half + 1):
                    o0 = [
                        ps.tile([128, NT], F32, tag="ps", name=f"o0_{t}_{kh}")
                        for kh in range(2)
                    ]
                    for c in range(NJ0):
                        z0 = zp0.tile([128, NT], F16, tag="z0")
                        nc.vector.tensor_mul(
                            z0,
                            featH[:, c * BD + t * NT : c * BD + (t + 1) * NT],
                            featR[:, t * NT : (t + 1) * NT],
                        )
                        for kh in range(2):
                            nc.tensor.matmul(
                                o0[kh],
                                w0[:, c * K + kh * 128 : c * K + (kh + 1) * 128],
                                z0,
                                start=(c == 0),
                                stop=(c == NJ0 - 1),
                            )
                    drain(o0[0], b0[:, 0:1], t, h1, None)
                    drain(o0[1], b0[:, 1:2], t, None, r0)

                # ---------------- Layers 1, 2 ----------------
                for lyr, (w_sb, h_in, b_sb) in enumerate(
                    [(w1, h1, b1), (w2, h2, b2)], start=1
                ):
                    o = [
                        [
                            ps.tile([128, NT], F32, tag="ps", name=f"o{lyr}_{half}_{kh}_{u}")
                            for u in range(2)
                        ]
                        for kh in range(2)
                    ]
                    for j in range(F0):
                        z = zp16.tile([128, HB], F16, tag="z")
                        nc.vector.tensor_mul(
                            z,
                            h_in[:, hoff : hoff + HB],
                            fbh[j // 16][:, (j % 16) * HB : (j % 16 + 1) * HB],
                        )
                        for kh in range(2):
                            wsl = w_sb[:, j * K + kh * 128 : j * K + (kh + 1) * 128]
                            for u in range(2):
                                nc.tensor.matmul(
                                    o[kh][u],
                                    wsl,
                                    z[:, u * NT : (u + 1) * NT],
                                    start=(j == 0),
                                    stop=(j == F0 - 1),
                                )
                    for u in range(2):
                        t = 2 * half + u
                        if lyr == 1:
                            drain(o[0][u], b_sb[:, 0:1], t, h2, None)
                            drain(o[1][u], b_sb[:, 1:2], t, None, r1)
                        else:
                            drain(o[0][u], b_sb[:, 0:1], t, None, r2a)
                            drain(o[1][u], b_sb[:, 1:2], t, None, r2b)

            nc.sync.dma_start(out_d[0:128, :], r0)
            nc.sync.dma_start(out_d[128:256, :], r1)
            nc.sync.dma_start(out_d[256:384, :], r2a)
            nc.sync.dma_start(out_d[384:512, :], r2b)

    nc.compile()
    return nc


def _host_prep(feat, W0, b0, W1, b1, W2, b2):
    """Rearrange full inputs into the per-core in_maps."""
    feat = np.ascontiguousarray(feat, dtype=np.float32)

    # W0: chunks c of 128 (i,j)-pairs, i-major: p = (i_local, j), i = 4c + p//32
    A = np.ascontiguousarray(W0.transpose(1, 2, 0)).reshape(F0 * F0, K)
    w0t = np.ascontiguousarray(
        A.reshape(NJ0, 128, K).transpose(1, 0, 2).reshape(128, NJ0 * K)
    ).astype(np.float16)
    w1t = np.ascontiguousarray(W1.transpose(1, 2, 0)).reshape(H, F0 * K).astype(np.float16)
    w2t = np.ascontiguousarray(W2.transpose(1, 2, 0)).reshape(H, F0 * K).astype(np.float16)

    b0t = np.ascontiguousarray(b0.reshape(2, 128).T).astype(np.float32)
    b1t = np.ascontiguousarray(b1.reshape(2, 128).T).astype(np.float32)
    b2t = np.ascontiguousarray(b2.reshape(2, 128).T).astype(np.float32)

    p = np.arange(128)
    in_maps = []
    for c in range(NCORES):
        fc = feat[c * BL : (c + 1) * BL]                        # [64, 32, 32]
        featT = np.ascontiguousarray(fc.transpose(1, 0, 2)).reshape(F0, BD)
        featT = featT.astype(np.float16)
        featR = np.ascontiguousarray(featT[p % F0])             # [128, BD]
        featH = np.concatenate(
            [featT[4 * cc + p // F0] for cc in range(NJ0)], axis=1
        )                                                        # [128, NJ0*BD]
        in_maps.append(
            {
                "featT16": featT,
                "featR": featR,
                "featH": np.ascontiguousarray(featH),
                "w0t": w0t,
                "w1t": w1t,
                "w2t": w2t,
                "b0t": b0t,
                "b1t": b1t,
                "b2t": b2t,
            }
        )
    return in_maps


def kernel(feat, W0, b0, W1, b1, W2, b2):
    global LAST_RESULTS
    if "nc" not in _CACHE:
        _CACHE["nc"] = _build_program()
    nc = _CACHE["nc"]
    in_maps = _host_prep(feat, W0, b0, W1, b1, W2, b2)
    res = run_bass_kernel_spmd(nc, in_maps, core_ids=list(range(NCORES)))
    LAST_RESULTS = res
    out = np.concatenate([res.results[c]["out"].T for c in range(NCORES)], axis=0)
    return np.ascontiguousarray(out, dtype=np.float32)


# revision 18
# speedup vs baseline: 325.0757x; 325.0757x over previous
"""CIN block kernel for Trainium2 (8 NeuronCores, data-parallel over batch).

Reference computation (per layer l, h0 = feat):
    out_l[b,k,d] = relu( sum_{i,j} W_l[k,i,j] * h_l[b,i,d] * feat[b,j,d] + b_l[k] )
    h_{l+1} = out_l[:, :K/2, :]   (split-half, except last layer)
    result  = concat([out0[:,128:], out1[:,128:], out2[:,:]], axis=1).sum(-1)

Mapping (per core, B_local=64, BD = B_local*D = 2048):
    Tensors live as [channel, (b,d)] with (b,d) flattened on the free dim.
    z_j[i, bd] = h[i, bd] * feat[j, bd]: feat row j is replicated across the
    128 partitions by a broadcast DMA from DRAM (src partition-stride 0) into
    a persistent fb buffer, reused by layers 1+2; z_j is one fp16 DVE
    multiply. out[k, bd] = sum_j Wt_j[i,k].T @ z_j: PE matmuls accumulating
    in PSUM (fp32), drained by the scalar engine as relu(x + b).
    Layer 0 (h = feat) contracts 1024 (i,j) pairs in 8 chunks of 128; both
    replicated factors (featH, featR) are host-prepared inputs, so layer 0 is
    one DVE multiply + matmuls per chunk.
    The batch is processed in two halves of 1024 positions so the 32 fb
    tiles (8 MB fp16) fit in SBUF. Everything is fp16 with fp32 accumulation.
"""

import os
import sys

import numpy as np

for _p in ("/opt/trn_rl_repo", "/root/.axon_site/_ro/trn_rl_repo"):
    if os.path.isdir(_p) and _p not in sys.path:
        sys.path.insert(0, _p)

import concourse.bacc as bacc
import concourse.bass as bass
import concourse.mybir as mybir
import concourse.tile as tile
from concourse.bass_utils import run_bass_kernel_spmd

F32 = mybir.dt.float32
F16 = mybir.dt.float16
RELU = mybir.ActivationFunctionType.Relu
AXX = mybir.AxisListType.X

NCORES = 8
B, F0, D = 512, 32, 32
BL = B // NCORES          # 64 batch rows per core
BD = BL * D               # 2048 free positions per core
NT = 512                  # free-dim tile (one PSUM bank)
HB = 1024                 # half of BD
K = 256                   # channels per layer
H = 128                   # hidden rows fed to layers 1,2 (split-half of 256)
NJ0 = F0 * F0 // 128      # 8 partition-chunks for layer-0 (i,j) pairs
T_TILES = BD // NT        # 4 bd-tiles

_CACHE = {}
LAST_RESULTS = None


def _build_program():
    nc = bacc.Bacc("TRN2", target_bir_lowering=False, debug=False)

    featT_d = nc.dram_tensor("featT16", [F0, BD], F16, kind="ExternalInput").ap()
    featR_d = nc.dram_tensor("featR", [128, BD], F16, kind="ExternalInput").ap()
    featH_d = nc.dram_tensor("featH", [128, NJ0 * BD], F16, kind="ExternalInput").ap()
    w0_d = nc.dram_tensor("w0t", [128, NJ0 * K], F16, kind="ExternalInput").ap()
    w1_d = nc.dram_tensor("w1t", [128, F0 * K], F16, kind="ExternalInput").ap()
    w2_d = nc.dram_tensor("w2t", [128, F0 * K], F16, kind="ExternalInput").ap()
    b0_d = nc.dram_tensor("b0t", [128, 2], F32, kind="ExternalInput").ap()
    b1_d = nc.dram_tensor("b1t", [128, 2], F32, kind="ExternalInput").ap()
    b2_d = nc.dram_tensor("b2t", [128, 2], F32, kind="ExternalInput").ap()
    out_d = nc.dram_tensor("out", [512, BL], F32, kind="ExternalOutput").ap()

    with tile.TileContext(nc) as tc:
        with (
            tc.tile_pool(name="const", bufs=1) as const,
            tc.tile_pool(name="ps", bufs=8, space="PSUM") as ps,
            tc.tile_pool(name="zp16", bufs=4) as zp16,
            tc.tile_pool(name="zp0", bufs=4) as zp0,
            tc.tile_pool(name="dp", bufs=3) as dp,
        ):
            # featH is t-major: col = t*(NJ0*NT) + c*NT + q; split DMAs so the
            # first layer-0 tile only waits on its own 1MB slice.
            featR = const.tile([128, BD], F16)
            featH = const.tile([128, NJ0 * BD], F16)
            for t in range(T_TILES):
                nc.sync.dma_start(
                    featR[:, t * NT : (t + 1) * NT], featR_d[:, t * NT : (t + 1) * NT]
                )
                nc.sync.dma_start(
                    featH[:, t * NJ0 * NT : (t + 1) * NJ0 * NT],
                    featH_d[:, t * NJ0 * NT : (t + 1) * NJ0 * NT],
                )
            w0 = const.tile([128, NJ0 * K], F16)
            nc.sync.dma_start(w0, w0_d)
            w1 = const.tile([128, F0 * K], F16)
            nc.sync.dma_start(w1, w1_d)
            w2 = const.tile([128, F0 * K], F16)
            nc.sync.dma_start(w2, w2_d)
            b0 = const.tile([128, 2], F32)
            nc.sync.dma_start(b0, b0_d)
            b1 = const.tile([128, 2], F32)
            nc.sync.dma_start(b1, b1_d)
            b2 = const.tile([128, 2], F32)
            nc.sync.dma_start(b2, b2_d)

            h1 = const.tile([128, BD], F16)
            h2 = const.tile([128, BD], F16)
            # 32 broadcast tiles for one half, split in two for cross-half overlap
            fbh = [const.tile([128, 16 * HB], F16, name=f"fbh{i}") for i in range(2)]
            r0 = const.tile([128, BL], F32)
            r1 = const.tile([128, BL], F32)
            r2a = const.tile([128, BL], F32)
            r2b = const.tile([128, BL], F32)

            def drain(o_ps, bias_ap, t, h_out, r_out):
                """relu(psum + bias) -> fp16 h slice, or f32 tile + d-reduce."""
                if h_out is not None:
                    nc.scalar.activation(
                        h_out[:, t * NT : (t + 1) * NT], o_ps, RELU, bias=bias_ap
                    )
                else:
                    dx = dp.tile([128, NT], F32, tag="d", name=f"d_{t}")
                    nc.scalar.activation(dx, o_ps, RELU, bias=bias_ap)
                    nc.vector.reduce_sum(
                        r_out[:, t * (NT // D) : (t + 1) * (NT // D)],
                        dx.rearrange("p (b d) -> p b d", d=D),
                        axis=AXX,
                    )

            for half in range(2):
                hoff = half * HB

                # fb prefetch: feat row j broadcast to 128 partitions via DMA
                for j in range(F0):
                    eng = nc.sync if j % 2 == 0 else nc.gpsimd
                    eng.dma_start(
                        fbh[j // 16][:, (j % 16) * HB : (j % 16 + 1) * HB],
                        featT_d[j : j + 1, hoff : hoff + HB].to_broadcast([128, HB]),
                    )

                # ---------------- Layer 0 (h = feat) ----------------
                for t in (2 * half, 2 * half + 1):
                    o0 = [
                        ps.tile([128, NT], F32, tag="ps", name=f"o0_{t}_{kh}")
                        for kh in range(2)
                    ]
                    for c in range(NJ0):
                        z0 = zp0.tile([128, NT], F16, tag="z0")
                        nc.vector.tensor_mul(
                            z0,
                            featH[:, (t * NJ0 + c) * NT : (t * NJ0 + c + 1) * NT],
                            featR[:, t * NT : (t + 1) * NT],
                        )
                        for kh in range(2):
                            nc.tensor.matmul(
                                o0[kh],
                                w0[:, c * K + kh * 128 : c * K + (kh + 1) * 128],
                                z0,
                                start=(c == 0),
                                stop=(c == NJ0 - 1),
                            )
                    drain(o0[0], b0[:, 0:1], t, h1, None)
                    drain(o0[1], b0[:, 1:2], t, None, r0)

                # ---------------- Layers 1, 2 ----------------
                for lyr, (w_sb, h_in, b_sb) in enumerate(
                    [(w1, h1, b1), (w2, h2, b2)], start=1
                ):
                    o = [
                        [
                            ps.tile([128, NT], F32, tag="ps", name=f"o{lyr}_{half}_{kh}_{u}")
                            for u in range(2)
                        ]
                        for kh in range(2)
                    ]
                    for j in range(F0):
                        z = zp16.tile([128, HB], F16, tag="z")
                        nc.vector.tensor_mul(
                            z,
                            h_in[:, hoff : hoff + HB],
                            fbh[j // 16][:, (j % 16) * HB : (j % 16 + 1) * HB],
                        )
                        for kh in range(2):
                            wsl = w_sb[:, j * K + kh * 128 : j * K + (kh + 1) * 128]
                            for u in range(2):
                                nc.tensor.matmul(
                                    o[kh][u],
                                    wsl,
                                    z[:, u * NT : (u + 1) * NT],
                                    start=(j == 0),
                                    stop=(j == F0 - 1),
                                )
                    for u in range(2):
                        t = 2 * half + u
                        if lyr == 1:
                            drain(o[0][u], b_sb[:, 0:1], t, h2, None)
                            drain(o[1][u], b_sb[:, 1:2], t, None, r1)
                        else:
                            drain(o[0][u], b_sb[:, 0:1], t, None, r2a)
                            drain(o[1][u], b_sb[:, 1:2], t, None, r2b)

            nc.sync.dma_start(out_d[0:128, :], r0)
            nc.sync.dma_start(out_d[128:256, :], r1)
            nc.sync.dma_start(out_d[256:384, :], r2a)
            nc.sync.dma_start(out_d[384:512, :], r2b)

    nc.compile()
    return nc


def _host_prep(feat, W0, b0, W1, b1, W2, b2):
    """Rearrange full inputs into the per-core in_maps."""
    feat = np.ascontiguousarray(feat, dtype=np.float32)

    # W0: chunks c of 128 (i,j)-pairs, i-major: p = (i_local, j), i = 4c + p//32
    A = np.ascontiguousarray(W0.transpose(1, 2, 0)).reshape(F0 * F0, K)
    w0t = np.ascontiguousarray(
        A.reshape(NJ0, 128, K).transpose(1, 0, 2).reshape(128, NJ0 * K)
    ).astype(np.float16)
    w1t = np.ascontiguousarray(W1.transpose(1, 2, 0)).reshape(H, F0 * K).astype(np.float16)
    w2t = np.ascontiguousarray(W2.transpose(1, 2, 0)).reshape(H, F0 * K).astype(np.float16)

    b0t = np.ascontiguousarray(b0.reshape(2, 128).T).astype(np.float32)
    b1t = np.ascontiguousarray(b1.reshape(2, 128).T).astype(np.float32)
    b2t = np.ascontiguousarray(b2.reshape(2, 128).T).astype(np.float32)

    p = np.arange(128)
    in_maps = []
    for c in range(NCORES):
        fc = feat[c * BL : (c + 1) * BL]                        # [64, 32, 32]
        featT = np.ascontiguousarray(fc.transpose(1, 0, 2)).reshape(F0, BD)
        featT = featT.astype(np.float16)
        featR = np.ascontiguousarray(featT[p % F0])             # [128, BD]
        featH = np.concatenate(
            [
                featT[4 * cc + p // F0, t * NT : (t + 1) * NT]
                for t in range(T_TILES)
                for cc in range(NJ0)
            ],
            axis=1,
        )                                                        # [128, NJ0*BD] t-major
        in_maps.append(
            {
                "featT16": featT,
                "featR": featR,
                "featH": np.ascontiguousarray(featH),
                "w0t": w0t,
                "w1t": w1t,
                "w2t": w2t,
                "b0t": b0t,
                "b1t": b1t,
                "b2t": b2t,
            }
        )
    return in_maps


def kernel(feat, W0, b0, W1, b1, W2, b2):
    global LAST_RESULTS
    if "nc" not in _CACHE:
        _CACHE["nc"] = _build_program()
    nc = _CACHE["nc"]
    in_maps = _host_prep(feat, W0, b0, W1, b1, W2, b2)
    res = run_bass_kernel_spmd(nc, in_maps, core_ids=list(range(NCORES)))
    LAST_RESULTS = res
    out = np.concatenate([res.results[c]["out"].T for c in range(NCORES)], axis=0)
    return np.ascontiguousarray(out, dtype=np.float32)


# revision 43
# speedup vs baseline: 11061.2873x; 34.0268x over previous
"""CIN block kernel for Trainium2 (8 NeuronCores, data-parallel over batch).

Reference computation (per layer l, h0 = feat):
    out_l[b,k,d] = relu( sum_{i,j} W_l[k,i,j] * h_l[b,i,d] * feat[b,j,d] + b_l[k] )
    h_{l+1} = out_l[:, :K/2, :]   (split-half, except last layer)
    result  = concat([out0[:,128:], out1[:,128:], out2[:,:]], axis=1).sum(-1)

Mapping (per core, B_local=64, BD = B_local*D = 2048):
    Tensors live as [channel, (b,d)] with (b,d) flattened on the free dim.
    z_j[i, bd] = h[i, bd] * feat[j, bd]: feat row j is replicated across the
    128 partitions by a broadcast DMA from DRAM (src partition-stride 0) into
    a persistent fb buffer, reused by layers 1+2; z_j is one fp16 DVE
    multiply. out[k, bd] = sum_j Wt_j[i,k].T @ z_j: PE matmuls accumulating
    in PSUM (fp32), drained by the scalar engine as relu(x + b).
    Layer 0 (h = feat) contracts 1024 (i,j) pairs in 8 chunks of 128; both
    replicated factors (featH, featR) are host-prepared inputs, so layer 0 is
    one DVE multiply + matmuls per chunk.
    The batch is processed in two halves of 1024 positions so the 32 fb
    tiles (8 MB fp16) fit in SBUF. Everything is fp16 with fp32 accumulation.
"""

import os
import sys

import numpy as np

for _p in ("/opt/trn_rl_repo", "/root/.axon_site/_ro/trn_rl_repo"):
    if os.path.isdir(_p) and _p not in sys.path:
        sys.path.insert(0, _p)

import concourse.bacc as bacc
import concourse.bass as bass
import concourse.mybir as mybir
import concourse.tile as tile
from concourse.bass_utils import run_bass_kernel_spmd

F32 = mybir.dt.float32
F16 = mybir.dt.float16
RELU = mybir.ActivationFunctionType.Relu
AXX = mybir.AxisListType.X

NCORES = 8
B, F0, D = 512, 32, 32
BL = B // NCORES          # 64 batch rows per core
BD = BL * D               # 2048 free positions per core
NT = 512                  # free-dim tile (one PSUM bank)
HB = 1024                 # half of BD
K = 256                   # channels per layer
H = 128                   # hidden rows fed to layers 1,2 (split-half of 256)
NJ0 = F0 * F0 // 128      # 8 partition-chunks for layer-0 (i,j) pairs
T_TILES = BD // NT        # 4 bd-tiles

_CACHE = {}
LAST_RESULTS = None


def _build_program(
    feath_splits=1,      # how many DMAs for featH/featR loads
    zp16_bufs=6,
    zp0_bufs=6,
    dp_bufs=4,
    fb_engines=("sync",),  # round-robin for fb broadcast DMAs
    ps_bufs=8,
    n_layers=3,          # for perf experiments only (output wrong if < 3)
    fb_mode="dma",       # "dma" | "shuffle" | "alt" (odd j on DVE stream_shuffle)
    feath_onchip=True,   # build layer-0 h-replica via PE selection matmuls
    reduce_on_act=False,  # d-sums via ACT activation accum_out instead of DVE
    hr_drain_act=False,   # drain layer-0 h-replica psum via ACT to fp16 SBUF
):
    nc = bacc.Bacc("TRN2", target_bir_lowering=False, debug=False)

    featT_d = nc.dram_tensor("featT16", [F0, BD], F16, kind="ExternalInput").ap()
    featR_d = nc.dram_tensor("featR", [128, BD], F16, kind="ExternalInput").ap()
    featH_d = nc.dram_tensor("featH", [128, NJ0 * BD], F16, kind="ExternalInput").ap()
    s4_d = nc.dram_tensor("s4all", [F0, NJ0 * 128], F16, kind="ExternalInput").ap()
    w0_d = nc.dram_tensor("w0t", [128, NJ0 * K], F16, kind="ExternalInput").ap()
    w1_d = nc.dram_tensor("w1t", [128, F0 * K], F16, kind="ExternalInput").ap()
    w2_d = nc.dram_tensor("w2t", [128, F0 * K], F16, kind="ExternalInput").ap()
    b0_d = nc.dram_tensor("b0t", [128, 2], F32, kind="ExternalInput").ap()
    b1_d = nc.dram_tensor("b1t", [128, 2], F32, kind="ExternalInput").ap()
    b2_d = nc.dram_tensor("b2t", [128, 2], F32, kind="ExternalInput").ap()
    out_d = nc.dram_tensor("out", [512, BL], F32, kind="ExternalOutput").ap()

    with tile.TileContext(nc) as tc:
        with (
            tc.tile_pool(name="const", bufs=1) as const,
            tc.tile_pool(name="ps", bufs=ps_bufs, space="PSUM") as ps,
            tc.tile_pool(name="zp16", bufs=zp16_bufs) as zp16,
            tc.tile_pool(name="zp0", bufs=zp0_bufs) as zp0,
            tc.tile_pool(name="dp", bufs=dp_bufs) as dp,
        ):
            # featH is t-major: col = t*(NJ0*NT) + c*NT + q; split DMAs so the
            # first layer-0 tile only waits on its own 1MB slice.
            featR = const.tile([128, BD], F16)
            sw = BD // feath_splits
            for s in range(feath_splits):
                nc.sync.dma_start(
                    featR[:, s * sw : (s + 1) * sw], featR_d[:, s * sw : (s + 1) * sw]
                )
            if feath_onchip in (True, "h0"):
                feat16 = const.tile([F0, BD], F16)
                nc.sync.dma_start(feat16, featT_d)
                s4 = const.tile([F0, NJ0 * 128], F16)
                nc.sync.dma_start(s4, s4_d)
            if feath_onchip is True:
                featH = None
            elif feath_onchip == "h0":
                # only the second half's slice of featH comes from DRAM
                featH = const.tile([128, NJ0 * BD], F16)
                nc.sync.dma_start(
                    featH[:, NJ0 * BD // 2 :], featH_d[:, NJ0 * BD // 2 :]
                )
            else:
                featH = const.tile([128, NJ0 * BD], F16)
                swh = NJ0 * BD // feath_splits
                for s in range(feath_splits):
                    nc.sync.dma_start(
                        featH[:, s * swh : (s + 1) * swh],
                        featH_d[:, s * swh : (s + 1) * swh],
                    )
            w0 = const.tile([128, NJ0 * K], F16)
            nc.sync.dma_start(w0, w0_d)
            w1 = const.tile([128, F0 * K], F16)
            nc.sync.dma_start(w1, w1_d)
            w2 = const.tile([128, F0 * K], F16)
            nc.sync.dma_start(w2, w2_d)
            b0 = const.tile([128, 2], F32)
            nc.sync.dma_start(b0, b0_d)
            b1 = const.tile([128, 2], F32)
            nc.sync.dma_start(b1, b1_d)
            b2 = const.tile([128, 2], F32)
            nc.sync.dma_start(b2, b2_d)

            h1 = const.tile([128, BD], F16)
            h2 = const.tile([128, BD], F16)
            # 32 broadcast tiles for one half, split in two for cross-half overlap
            fbh = [const.tile([128, 16 * HB], F16, name=f"fbh{i}") for i in range(2)]
            r0 = const.tile([128, BL], F32)
            r1 = const.tile([128, BL], F32)
            r2a = const.tile([128, BL], F32)
            r2b = const.tile([128, BL], F32)

            def drain(o_ps, bias_ap, t, h_out, r_out):
                """relu(psum + bias) -> fp16 h slice, or f32 tile + d-reduce."""
                if h_out is not None:
                    nc.scalar.activation(
                        h_out[:, t * NT : (t + 1) * NT], o_ps, RELU, bias=bias_ap
                    )
                elif reduce_on_act:
                    dx = dp.tile([128, NT], F32, tag="d", name=f"d_{t}")
                    for bb in range(NT // D):
                        nc.scalar.activation(
                            dx[:, bb * D : (bb + 1) * D],
                            o_ps[:, bb * D : (bb + 1) * D],
                            RELU,
                            bias=bias_ap,
                            accum_out=r_out[:, t * (NT // D) + bb : t * (NT // D) + bb + 1],
                        )
                else:
                    dx = dp.tile([128, NT], F32, tag="d", name=f"d_{t}")
                    nc.scalar.activation(dx, o_ps, RELU, bias=bias_ap)
                    nc.vector.reduce_sum(
                        r_out[:, t * (NT // D) : (t + 1) * (NT // D)],
                        dx.rearrange("p (b d) -> p b d", d=D),
                        axis=AXX,
                    )

            for half in range(2):
                hoff = half * HB

                # fb prefetch: feat row j broadcast to 128 partitions, either by
                # a DMA from DRAM (src partition-stride 0) or an on-chip DVE
                # stream_shuffle from featR (feat[p%32] -> mask [j]*32).
                for j in range(F0):
                    dst = fbh[j // 16][:, (j % 16) * HB : (j % 16 + 1) * HB]
                    use_shuffle = fb_mode == "shuffle" or (
                        fb_mode == "alt" and j % 2 == 1
                    )
                    if use_shuffle:
                        nc.vector.stream_shuffle(
                            dst, featR[:, hoff : hoff + HB], [j] * 32
                        )
                    else:
                        eng = getattr(nc, fb_engines[j % len(fb_engines)])
                        eng.dma_start(
                            dst,
                            featT_d[j : j + 1, hoff : hoff + HB].to_broadcast([128, HB]),
                        )

                # ---------------- Layer 0 (h = feat) ----------------
                for t in (2 * half, 2 * half + 1):
                    o0 = [
                        ps.tile([128, NT], F32, tag="ps", name=f"o0_{t}_{kh}")
                        for kh in range(2)
                    ]
                    for c in range(NJ0):
                        z0 = zp0.tile([128, NT], F16, tag="z0")
                        if feath_onchip is True or (feath_onchip == "h0" and half == 0):
                            hr_ps = ps.tile([128, NT], F32, tag="ps", name=f"hr_{t}_{c}")
                            nc.tensor.matmul(
                                hr_ps,
                                s4[:, c * 128 : (c + 1) * 128],
                                feat16[:, t * NT : (t + 1) * NT],
                                start=True,
                                stop=True,
                            )
                            if hr_drain_act:
                                hr16 = dp.tile([128, NT], F16, tag="hr16", name=f"hr16_{t}_{c}")
                                nc.scalar.copy(hr16, hr_ps)
                                nc.vector.tensor_mul(
                                    z0, hr16, featR[:, t * NT : (t + 1) * NT]
                                )
                            else:
                                nc.vector.tensor_mul(
                                    z0, hr_ps, featR[:, t * NT : (t + 1) * NT]
                                )
                        else:
                            nc.vector.tensor_mul(
                                z0,
                                featH[:, (t * NJ0 + c) * NT : (t * NJ0 + c + 1) * NT],
                                featR[:, t * NT : (t + 1) * NT],
                            )
                        for kh in range(2):
                            nc.tensor.matmul(
                                o0[kh],
                                w0[:, c * K + kh * 128 : c * K + (kh + 1) * 128],
                                z0,
                                start=(c == 0),
                                stop=(c == NJ0 - 1),
                            )
                    drain(o0[0], b0[:, 0:1], t, h1, None)
                    drain(o0[1], b0[:, 1:2], t, None, r0)

                # ---------------- Layers 1, 2 ----------------
                for lyr, (w_sb, h_in, b_sb) in enumerate(
                    [(w1, h1, b1), (w2, h2, b2)][: n_layers - 1], start=1
                ):
                    o = [
                        [
                            ps.tile([128, NT], F32, tag="ps", name=f"o{lyr}_{half}_{kh}_{u}")
                            for u in range(2)
                        ]
                        for kh in range(2)
                    ]
                    for j in range(F0):
                        z = zp16.tile([128, HB], F16, tag="z")
                        nc.vector.tensor_mul(
                            z,
                            h_in[:, hoff : hoff + HB],
                            fbh[j // 16][:, (j % 16) * HB : (j % 16 + 1) * HB],
                        )
                        for kh in range(2):
                            wsl = w_sb[:, j * K + kh * 128 : j * K + (kh + 1) * 128]
                            for u in range(2):
                                nc.tensor.matmul(
                                    o[kh][u],
                                    wsl,
                                    z[:, u * NT : (u + 1) * NT],
                                    start=(j == 0),
                                    stop=(j == F0 - 1),
                                )
                    for u in range(2):
                        t = 2 * half + u
                        if lyr == 1:
                            drain(o[0][u], b_sb[:, 0:1], t, h2, None)
                            drain(o[1][u], b_sb[:, 1:2], t, None, r1)
                        else:
                            drain(o[0][u], b_sb[:, 0:1], t, None, r2a)
                            drain(o[1][u], b_sb[:, 1:2], t, None, r2b)

            nc.sync.dma_start(out_d[0:128, :], r0)
            if n_layers >= 2:
                nc.sync.dma_start(out_d[128:256, :], r1)
            if n_layers >= 3:
                nc.sync.dma_start(out_d[256:384, :], r2a)
                nc.sync.dma_start(out_d[384:512, :], r2b)

    nc.compile()
    return nc


def _host_prep(feat, W0, b0, W1, b1, W2, b2):
    """Rearrange full inputs into the per-core in_maps."""
    feat = np.ascontiguousarray(feat, dtype=np.float32)

    # W0: chunks c of 128 (i,j)-pairs, i-major: p = (i_local, j), i = 4c + p//32
    A = np.ascontiguousarray(W0.transpose(1, 2, 0)).reshape(F0 * F0, K)
    w0t = np.ascontiguousarray(
        A.reshape(NJ0, 128, K).transpose(1, 0, 2).reshape(128, NJ0 * K)
    ).astype(np.float16)
    w1t = np.ascontiguousarray(W1.transpose(1, 2, 0)).reshape(H, F0 * K).astype(np.float16)
    w2t = np.ascontiguousarray(W2.transpose(1, 2, 0)).reshape(H, F0 * K).astype(np.float16)

    p_ = np.arange(128)
    s4all = np.zeros((F0, NJ0 * 128), np.float16)
    for cc in range(NJ0):
        s4all[:, cc * 128 : (cc + 1) * 128] = (
            (4 * cc + p_[None, :] // F0) == np.arange(F0)[:, None]
        )

    b0t = np.ascontiguousarray(b0.reshape(2, 128).T).astype(np.float32)
    b1t = np.ascontiguousarray(b1.reshape(2, 128).T).astype(np.float32)
    b2t = np.ascontiguousarray(b2.reshape(2, 128).T).astype(np.float32)

    p = np.arange(128)
    in_maps = []
    for c in range(NCORES):
        fc = feat[c * BL : (c + 1) * BL]                        # [64, 32, 32]
        featT = np.ascontiguousarray(fc.transpose(1, 0, 2)).reshape(F0, BD)
        featT = featT.astype(np.float16)
        featR = np.ascontiguousarray(featT[p % F0])             # [128, BD]
        featH = np.concatenate(
            [
                featT[4 * cc + p // F0, t * NT : (t + 1) * NT]
                for t in range(T_TILES)
                for cc in range(NJ0)
            ],
            axis=1,
        )                                                        # [128, NJ0*BD] t-major
        in_maps.append(
            {
                "featT16": featT,
                "featR": featR,
                "featH": np.ascontiguousarray(featH),
                "s4all": s4all,
                "w0t": w0t,
                "w1t": w1t,
                "w2t": w2t,
                "b0t": b0t,
                "b1t": b1t,
                "b2t": b2t,
            }
        )
    return in_maps


def kernel(feat, W0, b0, W1, b1, W2, b2):
    global LAST_RESULTS
    if "nc" not in _CACHE:
        _CACHE["nc"] = _build_program()
    nc = _CACHE["nc"]
    in_maps = _host_prep(feat, W0, b0, W1, b1, W2, b2)
    res = run_bass_kernel_spmd(nc, in_maps, core_ids=list(range(NCORES)))
    LAST_RESULTS = res
    out = np.concatenate([res.results[c]["out"].T for c in range(NCORES)], axis=0)
    return np.ascontiguousarray(out, dtype=np.float32)


# revision 54
# speedup vs baseline: 11435.1321x; 1.0338x over previous
"""CIN block kernel for Trainium2 (8 NeuronCores, data-parallel over batch).

Reference computation (per layer l, h0 = feat):
    out_l[b,k,d] = relu( sum_{i,j} W_l[k,i,j] * h_l[b,i,d] * feat[b,j,d] + b_l[k] )
    h_{l+1} = out_l[:, :K/2, :]   (split-half, except last layer)
    result  = concat([out0[:,128:], out1[:,128:], out2[:,:]], axis=1).sum(-1)

Mapping (per core, B_local=64, BD = B_local*D = 2048):
    Tensors live as [channel, (b,d)] with (b,d) flattened on the free dim.
    z_j[i, bd] = h[i, bd] * feat[j, bd]: feat row j is replicated across the
    128 partitions by a broadcast DMA from DRAM (src partition-stride 0) into
    a persistent fb buffer, reused by layers 1+2; z_j is one fp16 DVE
    multiply. out[k, bd] = sum_j Wt_j[i,k].T @ z_j: PE matmuls accumulating
    in PSUM (fp32), drained by the scalar engine as relu(x + b).
    Layer 0 (h = feat) contracts 1024 (i,j) pairs in 8 chunks of 128; both
    replicated factors (featH, featR) are host-prepared inputs, so layer 0 is
    one DVE multiply + matmuls per chunk.
    The batch is processed in two halves of 1024 positions so the 32 fb
    tiles (8 MB fp16) fit in SBUF. Everything is fp16 with fp32 accumulation.
"""

import os
import sys

import numpy as np

for _p in ("/opt/trn_rl_repo", "/root/.axon_site/_ro/trn_rl_repo"):
    if os.path.isdir(_p) and _p not in sys.path:
        sys.path.insert(0, _p)

import concourse.bacc as bacc
import concourse.bass as bass
import concourse.mybir as mybir
import concourse.tile as tile
from concourse.bass_utils import run_bass_kernel_spmd

F32 = mybir.dt.float32
F16 = mybir.dt.float16
RELU = mybir.ActivationFunctionType.Relu
AXX = mybir.AxisListType.X

NCORES = 8
B, F0, D = 512, 32, 32
BL = B // NCORES          # 64 batch rows per core
BD = BL * D               # 2048 free positions per core
NT = 512                  # free-dim tile (one PSUM bank)
HB = 1024                 # half of BD
K = 256                   # channels per layer
H = 128                   # hidden rows fed to layers 1,2 (split-half of 256)
NJ0 = F0 * F0 // 128      # 8 partition-chunks for layer-0 (i,j) pairs
T_TILES = BD // NT        # 4 bd-tiles

_CACHE = {}
LAST_RESULTS = None


def _build_program(
    feath_splits=1,      # how many DMAs for featH/featR loads
    zp16_bufs=6,
    zp0_bufs=6,
    dp_bufs=4,
    fb_engines=("sync",),  # round-robin for fb broadcast DMAs
    ps_bufs=8,
    n_layers=3,          # for perf experiments only (output wrong if < 3)
    fb_mode="dma",       # "dma" | "shuffle" | "alt" (odd j on DVE stream_shuffle)
    feath_onchip=True,   # build layer-0 h-replica via PE selection matmuls
    reduce_on_act=False,  # d-sums via ACT activation accum_out instead of DVE
    hr_drain_act=False,   # drain layer-0 h-replica psum via ACT to fp16 SBUF
    l0_early=True,        # emit half-1's layer 0 before half-0's layers 1/2
    h_drain_first=False,  # at layer end, drain h-producing banks before d banks
    fb_splits=2,          # fbh buffer count (2 or 3) for cross-half overlap
):
    nc = bacc.Bacc("TRN2", target_bir_lowering=False, debug=False)

    featT_d = nc.dram_tensor("featT16", [F0, BD], F16, kind="ExternalInput").ap()
    featR_d = nc.dram_tensor("featR", [128, BD], F16, kind="ExternalInput").ap()
    featH_d = nc.dram_tensor("featH", [128, NJ0 * BD], F16, kind="ExternalInput").ap()
    s4_d = nc.dram_tensor("s4all", [F0, NJ0 * 128], F16, kind="ExternalInput").ap()
    w0_d = nc.dram_tensor("w0t", [128, NJ0 * K], F16, kind="ExternalInput").ap()
    w1_d = nc.dram_tensor("w1t", [128, F0 * K], F16, kind="ExternalInput").ap()
    w2_d = nc.dram_tensor("w2t", [128, F0 * K], F16, kind="ExternalInput").ap()
    b0_d = nc.dram_tensor("b0t", [128, 2], F32, kind="ExternalInput").ap()
    b1_d = nc.dram_tensor("b1t", [128, 2], F32, kind="ExternalInput").ap()
    b2_d = nc.dram_tensor("b2t", [128, 2], F32, kind="ExternalInput").ap()
    out_d = nc.dram_tensor("out", [512, BL], F32, kind="ExternalOutput").ap()

    with tile.TileContext(nc) as tc:
        with (
            tc.tile_pool(name="const", bufs=1) as const,
            tc.tile_pool(name="ps", bufs=ps_bufs, space="PSUM") as ps,
            tc.tile_pool(name="zp16", bufs=zp16_bufs) as zp16,
            tc.tile_pool(name="zp0", bufs=zp0_bufs) as zp0,
            tc.tile_pool(name="dp", bufs=dp_bufs) as dp,
        ):
            # featH is t-major: col = t*(NJ0*NT) + c*NT + q; split DMAs so the
            # first layer-0 tile only waits on its own 1MB slice.
            featR = const.tile([128, BD], F16)
            sw = BD // feath_splits
            for s in range(feath_splits):
                nc.sync.dma_start(
                    featR[:, s * sw : (s + 1) * sw], featR_d[:, s * sw : (s + 1) * sw]
                )
            if feath_onchip in (True, "h0"):
                feat16 = const.tile([F0, BD], F16)
                nc.sync.dma_start(feat16, featT_d)
                s4 = const.tile([F0, NJ0 * 128], F16)
                nc.sync.dma_start(s4, s4_d)
            if feath_onchip is True:
                featH = None
            elif feath_onchip == "h0":
                # only the second half's slice of featH comes from DRAM
                featH = const.tile([128, NJ0 * BD], F16)
                nc.sync.dma_start(
                    featH[:, NJ0 * BD // 2 :], featH_d[:, NJ0 * BD // 2 :]
                )
            else:
                featH = const.tile([128, NJ0 * BD], F16)
                swh = NJ0 * BD // feath_splits
                for s in range(feath_splits):
                    nc.sync.dma_start(
                        featH[:, s * swh : (s + 1) * swh],
                        featH_d[:, s * swh : (s + 1) * swh],
                    )
            w0 = const.tile([128, NJ0 * K], F16)
            nc.sync.dma_start(w0, w0_d)
            w1 = const.tile([128, F0 * K], F16)
            nc.sync.dma_start(w1, w1_d)
            w2 = const.tile([128, F0 * K], F16)
            nc.sync.dma_start(w2, w2_d)
            b0 = const.tile([128, 2], F32)
            nc.sync.dma_start(b0, b0_d)
            b1 = const.tile([128, 2], F32)
            nc.sync.dma_start(b1, b1_d)
            b2 = const.tile([128, 2], F32)
            nc.sync.dma_start(b2, b2_d)

            h1 = const.tile([128, BD], F16)
            h2 = const.tile([128, BD], F16)
            # 32 broadcast tiles for one half, split for cross-half overlap.
            # With 3 buffers, half-1's first group gets a fresh buffer, so its
            # DMAs can be emitted (and run) during half-0's layers 1/2.
            fb_grp = F0 // 2
            fbh = [
                const.tile([128, fb_grp * HB], F16, name=f"fbh{i}")
                for i in range(fb_splits)
            ]

            def fb_buf(half, j):
                return fbh[(2 * half + j // fb_grp) % fb_splits]
            r0 = const.tile([128, BL], F32)
            r1 = const.tile([128, BL], F32)
            r2a = const.tile([128, BL], F32)
            r2b = const.tile([128, BL], F32)

            def drain(o_ps, bias_ap, t, h_out, r_out):
                """relu(psum + bias) -> fp16 h slice, or f32 tile + d-reduce."""
                if h_out is not None:
                    nc.scalar.activation(
                        h_out[:, t * NT : (t + 1) * NT], o_ps, RELU, bias=bias_ap
                    )
                elif reduce_on_act:
                    dx = dp.tile([128, NT], F32, tag="d", name=f"d_{t}")
                    for bb in range(NT // D):
                        nc.scalar.activation(
                            dx[:, bb * D : (bb + 1) * D],
                            o_ps[:, bb * D : (bb + 1) * D],
                            RELU,
                            bias=bias_ap,
                            accum_out=r_out[:, t * (NT // D) + bb : t * (NT // D) + bb + 1],
                        )
                else:
                    dx = dp.tile([128, NT], F32, tag="d", name=f"d_{t}")
                    nc.scalar.activation(dx, o_ps, RELU, bias=bias_ap)
                    nc.vector.reduce_sum(
                        r_out[:, t * (NT // D) : (t + 1) * (NT // D)],
                        dx.rearrange("p (b d) -> p b d", d=D),
                        axis=AXX,
                    )

            def emit_fb(half, js=range(F0)):
                hoff = half * HB
                # fb prefetch: feat row j broadcast to 128 partitions, either by
                # a DMA from DRAM (src partition-stride 0) or an on-chip DVE
                # stream_shuffle from featR (feat[p%32] -> mask [j]*32).
                for j in js:
                    dst = fb_buf(half, j)[:, (j % fb_grp) * HB : (j % fb_grp + 1) * HB]
                    use_shuffle = fb_mode == "shuffle" or (
                        fb_mode == "alt" and j % 2 == 1
                    )
                    if use_shuffle:
                        nc.vector.stream_shuffle(
                            dst, featR[:, hoff : hoff + HB], [j] * 32
                        )
                    else:
                        eng = getattr(nc, fb_engines[j % len(fb_engines)])
                        eng.dma_start(
                            dst,
                            featT_d[j : j + 1, hoff : hoff + HB].to_broadcast([128, HB]),
                        )

            def emit_l0(half):
                # ---------------- Layer 0 (h = feat) ----------------
                for t in (2 * half, 2 * half + 1):
                    o0 = [
                        ps.tile([128, NT], F32, tag="ps", name=f"o0_{t}_{kh}")
                        for kh in range(2)
                    ]
                    for c in range(NJ0):
                        z0 = zp0.tile([128, NT], F16, tag="z0")
                        if feath_onchip is True or (feath_onchip == "h0" and half == 0):
                            hr_ps = ps.tile([128, NT], F32, tag="ps", name=f"hr_{t}_{c}")
                            nc.tensor.matmul(
                                hr_ps,
                                s4[:, c * 128 : (c + 1) * 128],
                                feat16[:, t * NT : (t + 1) * NT],
                                start=True,
                                stop=True,
                            )
                            if hr_drain_act:
                                hr16 = dp.tile([128, NT], F16, tag="hr16", name=f"hr16_{t}_{c}")
                                nc.scalar.copy(hr16, hr_ps)
                                nc.vector.tensor_mul(
                                    z0, hr16, featR[:, t * NT : (t + 1) * NT]
                                )
                            else:
                                nc.vector.tensor_mul(
                                    z0, hr_ps, featR[:, t * NT : (t + 1) * NT]
                                )
                        else:
                            nc.vector.tensor_mul(
                                z0,
                                featH[:, (t * NJ0 + c) * NT : (t * NJ0 + c + 1) * NT],
                                featR[:, t * NT : (t + 1) * NT],
                            )
                        for kh in range(2):
                            nc.tensor.matmul(
                                o0[kh],
                                w0[:, c * K + kh * 128 : c * K + (kh + 1) * 128],
                                z0,
                                start=(c == 0),
                                stop=(c == NJ0 - 1),
                            )
                    drain(o0[0], b0[:, 0:1], t, h1, None)
                    drain(o0[1], b0[:, 1:2], t, None, r0)

            def emit_l12(half):
                hoff = half * HB
                # ---------------- Layers 1, 2 ----------------
                for lyr, (w_sb, h_in, b_sb) in enumerate(
                    [(w1, h1, b1), (w2, h2, b2)][: n_layers - 1], start=1
                ):
                    o = [
                        [
                            ps.tile([128, NT], F32, tag="ps", name=f"o{lyr}_{half}_{kh}_{u}")
                            for u in range(2)
                        ]
                        for kh in range(2)
                    ]
                    for j in range(F0):
                        z = zp16.tile([128, HB], F16, tag="z")
                        nc.vector.tensor_mul(
                            z,
                            h_in[:, hoff : hoff + HB],
                            fb_buf(half, j)[:, (j % fb_grp) * HB : (j % fb_grp + 1) * HB],
                        )
                        for kh in range(2):
                            wsl = w_sb[:, j * K + kh * 128 : j * K + (kh + 1) * 128]
                            for u in range(2):
                                nc.tensor.matmul(
                                    o[kh][u],
                                    wsl,
                                    z[:, u * NT : (u + 1) * NT],
                                    start=(j == 0),
                                    stop=(j == F0 - 1),
                                )
                    if lyr == 1 and h_drain_first:
                        for u in range(2):
                            drain(o[0][u], b_sb[:, 0:1], 2 * half + u, h2, None)
                        for u in range(2):
                            drain(o[1][u], b_sb[:, 1:2], 2 * half + u, None, r1)
                    else:
                        for u in range(2):
                            t = 2 * half + u
                            if lyr == 1:
                                drain(o[0][u], b_sb[:, 0:1], t, h2, None)
                                drain(o[1][u], b_sb[:, 1:2], t, None, r1)
                            else:
                                drain(o[0][u], b_sb[:, 0:1], t, None, r2a)
                                drain(o[1][u], b_sb[:, 1:2], t, None, r2b)

            if l0_early:
                emit_fb(0)
                emit_l0(0)
                emit_l0(1)
                if fb_splits == 3:
                    emit_fb(1, range(fb_grp))       # fresh buffer, no WAR
                    emit_l12(0)
                    emit_fb(1, range(fb_grp, F0))   # reuses fbh[0] after half-0
                else:
                    emit_l12(0)
                    emit_fb(1)
                emit_l12(1)
            else:
                for half in range(2):
                    emit_fb(half)
                    emit_l0(half)
                    emit_l12(half)

            nc.sync.dma_start(out_d[0:128, :], r0)
            if n_layers >= 2:
                nc.sync.dma_start(out_d[128:256, :], r1)
            if n_layers >= 3:
                nc.sync.dma_start(out_d[256:384, :], r2a)
                nc.sync.dma_start(out_d[384:512, :], r2b)

    nc.compile()
    return nc


def _host_prep(feat, W0, b0, W1, b1, W2, b2):
    """Rearrange full inputs into the per-core in_maps."""
    feat = np.ascontiguousarray(feat, dtype=np.float32)

    # W0: chunks c of 128 (i,j)-pairs, i-major: p = (i_local, j), i = 4c + p//32
    A = np.ascontiguousarray(W0.transpose(1, 2, 0)).reshape(F0 * F0, K)
    w0t = np.ascontiguousarray(
        A.reshape(NJ0, 128, K).transpose(1, 0, 2).reshape(128, NJ0 * K)
    ).astype(np.float16)
    w1t = np.ascontiguousarray(W1.transpose(1, 2, 0)).reshape(H, F0 * K).astype(np.float16)
    w2t = np.ascontiguousarray(W2.transpose(1, 2, 0)).reshape(H, F0 * K).astype(np.float16)

    p_ = np.arange(128)
    s4all = np.zeros((F0, NJ0 * 128), np.float16)
    for cc in range(NJ0):
        s4all[:, cc * 128 : (cc + 1) * 128] = (
            (4 * cc + p_[None, :] // F0) == np.arange(F0)[:, None]
        )

    b0t = np.ascontiguousarray(b0.reshape(2, 128).T).astype(np.float32)
    b1t = np.ascontiguousarray(b1.reshape(2, 128).T).astype(np.float32)
    b2t = np.ascontiguousarray(b2.reshape(2, 128).T).astype(np.float32)

    p = np.arange(128)
    in_maps = []
    for c in range(NCORES):
        fc = feat[c * BL : (c + 1) * BL]                        # [64, 32, 32]
        featT = np.ascontiguousarray(fc.transpose(1, 0, 2)).reshape(F0, BD)
        featT = featT.astype(np.float16)
        featR = np.ascontiguousarray(featT[p % F0])             # [128, BD]
        featH = np.concatenate(
            [
                featT[4 * cc + p // F0, t * NT : (t + 1) * NT]
                for t in range(T_TILES)
                for cc in range(NJ0)
            ],
            axis=1,
        )                                                        # [128, NJ0*BD] t-major
        in_maps.append(
            {
                "featT16": featT,
                "featR": featR,
                "featH": np.ascontiguousarray(featH),
                "s4all": s4all,
                "w0t": w0t,
                "w1t": w1t,
                "w2t": w2t,
                "b0t": b0t,
                "b1t": b1t,
                "b2t": b2t,
            }
        )
    return in_maps


def kernel(feat, W0, b0, W1, b1, W2, b2):
    global LAST_RESULTS
    if "nc" not in _CACHE:
        _CACHE["nc"] = _build_program()
    nc = _CACHE["nc"]
    in_maps = _host_prep(feat, W0, b0, W1, b1, W2, b2)
    res = run_bass_kernel_spmd(nc, in_maps, core_ids=list(range(NCORES)))
    LAST_RESULTS = res
    out = np.concatenate([res.results[c]["out"].T for c in range(NCORES)], axis=0)
    return np.ascontiguousarray(out, dtype=np.float32)


# revision 61
# speedup vs baseline: 11531.4337x; 1.0084x over previous
"""CIN block kernel for Trainium2 (8 NeuronCores, data-parallel over batch).

Reference computation (per layer l, h0 = feat):
    out_l[b,k,d] = relu( sum_{i,j} W_l[k,i,j] * h_l[b,i,d] * feat[b,j,d] + b_l[k] )
    h_{l+1} = out_l[:, :K/2, :]   (split-half, except last layer)
    result  = concat([out0[:,128:], out1[:,128:], out2[:,:]], axis=1).sum(-1)

Mapping (per core, B_local=64, BD = B_local*D = 2048):
    Tensors live as [channel, (b,d)] with (b,d) flattened on the free dim.
    z_j[i, bd] = h[i, bd] * feat[j, bd]: feat row j is replicated across the
    128 partitions by a broadcast DMA from DRAM (src partition-stride 0) into
    a persistent fb buffer, reused by layers 1+2; z_j is one fp16 DVE
    multiply. out[k, bd] = sum_j Wt_j[i,k].T @ z_j: PE matmuls accumulating
    in PSUM (fp32), drained by the scalar engine as relu(x + b).
    Layer 0 (h = feat) contracts 1024 (i,j) pairs in 8 chunks of 128; both
    replicated factors (featH, featR) are host-prepared inputs, so layer 0 is
    one DVE multiply + matmuls per chunk.
    The batch is processed in two halves of 1024 positions so the 32 fb
    tiles (8 MB fp16) fit in SBUF. Everything is fp16 with fp32 accumulation.
"""

import os
import sys

import numpy as np

for _p in ("/opt/trn_rl_repo", "/root/.axon_site/_ro/trn_rl_repo"):
    if os.path.isdir(_p) and _p not in sys.path:
        sys.path.insert(0, _p)

import concourse.bacc as bacc
import concourse.bass as bass
import concourse.mybir as mybir
import concourse.tile as tile
from concourse.bass_utils import run_bass_kernel_spmd

F32 = mybir.dt.float32
F16 = mybir.dt.float16
RELU = mybir.ActivationFunctionType.Relu
AXX = mybir.AxisListType.X

NCORES = 8
B, F0, D = 512, 32, 32
BL = B // NCORES          # 64 batch rows per core
BD = BL * D               # 2048 free positions per core
NT = 512                  # free-dim tile (one PSUM bank)
HB = 1024                 # half of BD
K = 256                   # channels per layer
H = 128                   # hidden rows fed to layers 1,2 (split-half of 256)
NJ0 = F0 * F0 // 128      # 8 partition-chunks for layer-0 (i,j) pairs
T_TILES = BD // NT        # 4 bd-tiles

_CACHE = {}
LAST_RESULTS = None


def _build_program(
    feath_splits=1,      # how many DMAs for featH/featR loads
    zp16_bufs=6,
    zp0_bufs=6,
    dp_bufs=4,
    fb_engines=("sync",),  # round-robin for fb broadcast DMAs
    ps_bufs=8,
    n_layers=3,          # for perf experiments only (output wrong if < 3)
    fb_mode="dma",       # "dma" | "shuffle" | "alt" (odd j on DVE stream_shuffle)
    feath_onchip=True,   # build layer-0 h-replica via PE selection matmuls
    reduce_on_act=False,  # d-sums via ACT activation accum_out instead of DVE
    hr_drain_act=False,   # drain layer-0 h-replica psum via ACT to fp16 SBUF
    l0_early=True,        # emit half-1's layer 0 before half-0's layers 1/2
    h_drain_first=False,  # at layer end, drain h-producing banks before d banks
    fb_splits=2,          # fbh buffer count (2 or 3) for cross-half overlap
    z0_gpsimd=False,      # layer-0 multiply on GPSIMD (implies hr via ACT to SBUF)
    out_dma_split=False,  # emit output DMAs per half instead of at the end
    warmup_mms=4,         # dummy matmuls at t=0 to exit the HAM cold clock-gate
):
    nc = bacc.Bacc("TRN2", target_bir_lowering=False, debug=False)

    featT_d = nc.dram_tensor("featT16", [F0, BD], F16, kind="ExternalInput").ap()
    featR_d = nc.dram_tensor("featR", [128, BD], F16, kind="ExternalInput").ap()
    featH_d = nc.dram_tensor("featH", [128, NJ0 * BD], F16, kind="ExternalInput").ap()
    s4_d = nc.dram_tensor("s4all", [F0, NJ0 * 128], F16, kind="ExternalInput").ap()
    w0_d = nc.dram_tensor("w0t", [128, NJ0 * K], F16, kind="ExternalInput").ap()
    w1_d = nc.dram_tensor("w1t", [128, F0 * K], F16, kind="ExternalInput").ap()
    w2_d = nc.dram_tensor("w2t", [128, F0 * K], F16, kind="ExternalInput").ap()
    b0_d = nc.dram_tensor("b0t", [128, 2], F32, kind="ExternalInput").ap()
    b1_d = nc.dram_tensor("b1t", [128, 2], F32, kind="ExternalInput").ap()
    b2_d = nc.dram_tensor("b2t", [128, 2], F32, kind="ExternalInput").ap()
    out_d = nc.dram_tensor("out", [512, BL], F32, kind="ExternalOutput").ap()

    with tile.TileContext(nc) as tc:
        with (
            tc.tile_pool(name="const", bufs=1) as const,
            tc.tile_pool(name="ps", bufs=ps_bufs, space="PSUM") as ps,
            tc.tile_pool(name="zp16", bufs=zp16_bufs) as zp16,
            tc.tile_pool(name="zp0", bufs=zp0_bufs) as zp0,
            tc.tile_pool(name="dp", bufs=dp_bufs) as dp,
        ):
            # featH is t-major: col = t*(NJ0*NT) + c*NT + q; split DMAs so the
            # first layer-0 tile only waits on its own 1MB slice.
            if warmup_mms:
                # PE sits idle during the initial DMA loads; spend that window
                # on throwaway matmuls so the HAM clock-gate reaches 8/8
                # before the first real matmul issues.
                wt = const.tile([128, NT], F16, name="warm_sb")
                nc.vector.memset(wt, 0.0)
                wps = ps.tile([128, NT], F32, tag="ps", name="warm_ps")
                for _ in range(warmup_mms):
                    nc.tensor.matmul(wps, wt[:, :128], wt, start=True, stop=True)

            featR = const.tile([128, BD], F16)
            sw = BD // feath_splits
            for s in range(feath_splits):
                nc.sync.dma_start(
                    featR[:, s * sw : (s + 1) * sw], featR_d[:, s * sw : (s + 1) * sw]
                )
            if feath_onchip in (True, "h0"):
                feat16 = const.tile([F0, BD], F16)
                nc.sync.dma_start(feat16, featT_d)
                s4 = const.tile([F0, NJ0 * 128], F16)
                nc.sync.dma_start(s4, s4_d)
            if feath_onchip is True:
                featH = None
            elif feath_onchip == "h0":
                # only the second half's slice of featH comes from DRAM
                featH = const.tile([128, NJ0 * BD], F16)
                nc.sync.dma_start(
                    featH[:, NJ0 * BD // 2 :], featH_d[:, NJ0 * BD // 2 :]
                )
            else:
                featH = const.tile([128, NJ0 * BD], F16)
                swh = NJ0 * BD // feath_splits
                for s in range(feath_splits):
                    nc.sync.dma_start(
                        featH[:, s * swh : (s + 1) * swh],
                        featH_d[:, s * swh : (s + 1) * swh],
                    )
            w0 = const.tile([128, NJ0 * K], F16)
            nc.sync.dma_start(w0, w0_d)
            w1 = const.tile([128, F0 * K], F16)
            nc.sync.dma_start(w1, w1_d)
            w2 = const.tile([128, F0 * K], F16)
            nc.sync.dma_start(w2, w2_d)
            b0 = const.tile([128, 2], F32)
            nc.sync.dma_start(b0, b0_d)
            b1 = const.tile([128, 2], F32)
            nc.sync.dma_start(b1, b1_d)
            b2 = const.tile([128, 2], F32)
            nc.sync.dma_start(b2, b2_d)

            h1 = const.tile([128, BD], F16)
            h2 = const.tile([128, BD], F16)
            # 32 broadcast tiles for one half, split for cross-half overlap.
            # With 3 buffers, half-1's first group gets a fresh buffer, so its
            # DMAs can be emitted (and run) during half-0's layers 1/2.
            fb_grp = F0 // 2
            fbh = [
                const.tile([128, fb_grp * HB], F16, name=f"fbh{i}")
                for i in range(fb_splits)
            ]

            def fb_buf(half, j):
                return fbh[(2 * half + j // fb_grp) % fb_splits]
            r0 = const.tile([128, BL], F32)
            r1 = const.tile([128, BL], F32)
            r2a = const.tile([128, BL], F32)
            r2b = const.tile([128, BL], F32)

            def drain(o_ps, bias_ap, t, h_out, r_out):
                """relu(psum + bias) -> fp16 h slice, or f32 tile + d-reduce."""
                if h_out is not None:
                    nc.scalar.activation(
                        h_out[:, t * NT : (t + 1) * NT], o_ps, RELU, bias=bias_ap
                    )
                elif reduce_on_act:
                    dx = dp.tile([128, NT], F32, tag="d", name=f"d_{t}")
                    for bb in range(NT // D):
                        nc.scalar.activation(
                            dx[:, bb * D : (bb + 1) * D],
                            o_ps[:, bb * D : (bb + 1) * D],
                            RELU,
                            bias=bias_ap,
                            accum_out=r_out[:, t * (NT // D) + bb : t * (NT // D) + bb + 1],
                        )
                else:
                    dx = dp.tile([128, NT], F32, tag="d", name=f"d_{t}")
                    nc.scalar.activation(dx, o_ps, RELU, bias=bias_ap)
                    nc.vector.reduce_sum(
                        r_out[:, t * (NT // D) : (t + 1) * (NT // D)],
                        dx.rearrange("p (b d) -> p b d", d=D),
                        axis=AXX,
                    )

            def emit_fb(half, js=range(F0)):
                hoff = half * HB
                # fb prefetch: feat row j broadcast to 128 partitions, either by
                # a DMA from DRAM (src partition-stride 0) or an on-chip DVE
                # stream_shuffle from featR (feat[p%32] -> mask [j]*32).
                for j in js:
                    dst = fb_buf(half, j)[:, (j % fb_grp) * HB : (j % fb_grp + 1) * HB]
                    use_shuffle = fb_mode == "shuffle" or (
                        fb_mode == "alt" and j % 2 == 1
                    )
                    if use_shuffle:
                        nc.vector.stream_shuffle(
                            dst, featR[:, hoff : hoff + HB], [j] * 32
                        )
                    else:
                        eng = getattr(nc, fb_engines[j % len(fb_engines)])
                        eng.dma_start(
                            dst,
                            featT_d[j : j + 1, hoff : hoff + HB].to_broadcast([128, HB]),
                        )

            def emit_l0(half):
                # ---------------- Layer 0 (h = feat) ----------------
                for t in (2 * half, 2 * half + 1):
                    o0 = [
                        ps.tile([128, NT], F32, tag="ps", name=f"o0_{t}_{kh}")
                        for kh in range(2)
                    ]
                    for c in range(NJ0):
                        z0 = zp0.tile([128, NT], F16, tag="z0")
                        if feath_onchip is True or (feath_onchip == "h0" and half == 0):
                            hr_ps = ps.tile([128, NT], F32, tag="ps", name=f"hr_{t}_{c}")
                            nc.tensor.matmul(
                                hr_ps,
                                s4[:, c * 128 : (c + 1) * 128],
                                feat16[:, t * NT : (t + 1) * NT],
                                start=True,
                                stop=True,
                            )
                            if hr_drain_act or z0_gpsimd:
                                hr16 = dp.tile([128, NT], F16, tag="hr16", name=f"hr16_{t}_{c}")
                                nc.scalar.copy(hr16, hr_ps)
                                eng = nc.gpsimd if z0_gpsimd else nc.vector
                                eng.tensor_mul(
                                    z0, hr16, featR[:, t * NT : (t + 1) * NT]
                                )
                            else:
                                nc.vector.tensor_mul(
                                    z0, hr_ps, featR[:, t * NT : (t + 1) * NT]
                                )
                        else:
                            nc.vector.tensor_mul(
                                z0,
                                featH[:, (t * NJ0 + c) * NT : (t * NJ0 + c + 1) * NT],
                                featR[:, t * NT : (t + 1) * NT],
                            )
                        for kh in range(2):
                            nc.tensor.matmul(
                                o0[kh],
                                w0[:, c * K + kh * 128 : c * K + (kh + 1) * 128],
                                z0,
                                start=(c == 0),
                                stop=(c == NJ0 - 1),
                            )
                    drain(o0[0], b0[:, 0:1], t, h1, None)
                    drain(o0[1], b0[:, 1:2], t, None, r0)

            def emit_l12(half):
                hoff = half * HB
                # ---------------- Layers 1, 2 ----------------
                for lyr, (w_sb, h_in, b_sb) in enumerate(
                    [(w1, h1, b1), (w2, h2, b2)][: n_layers - 1], start=1
                ):
                    o = [
                        [
                            ps.tile([128, NT], F32, tag="ps", name=f"o{lyr}_{half}_{kh}_{u}")
                            for u in range(2)
                        ]
                        for kh in range(2)
                    ]
                    for j in range(F0):
                        z = zp16.tile([128, HB], F16, tag="z")
                        nc.vector.tensor_mul(
                            z,
                            h_in[:, hoff : hoff + HB],
                            fb_buf(half, j)[:, (j % fb_grp) * HB : (j % fb_grp + 1) * HB],
                        )
                        for kh in range(2):
                            wsl = w_sb[:, j * K + kh * 128 : j * K + (kh + 1) * 128]
                            for u in range(2):
                                nc.tensor.matmul(
                                    o[kh][u],
                                    wsl,
                                    z[:, u * NT : (u + 1) * NT],
                                    start=(j == 0),
                                    stop=(j == F0 - 1),
                                )
                    if lyr == 1 and h_drain_first:
                        for u in range(2):
                            drain(o[0][u], b_sb[:, 0:1], 2 * half + u, h2, None)
                        for u in range(2):
                            drain(o[1][u], b_sb[:, 1:2], 2 * half + u, None, r1)
                    else:
                        for u in range(2):
                            t = 2 * half + u
                            if lyr == 1:
                                drain(o[0][u], b_sb[:, 0:1], t, h2, None)
                                drain(o[1][u], b_sb[:, 1:2], t, None, r1)
                            else:
                                drain(o[0][u], b_sb[:, 0:1], t, None, r2a)
                                drain(o[1][u], b_sb[:, 1:2], t, None, r2b)

            def emit_out(half):
                cs = slice(half * BL // 2, (half + 1) * BL // 2)
                nc.sync.dma_start(out_d[0:128, cs], r0[:, cs])
                if n_layers >= 2:
                    nc.sync.dma_start(out_d[128:256, cs], r1[:, cs])
                if n_layers >= 3:
                    nc.sync.dma_start(out_d[256:384, cs], r2a[:, cs])
                    nc.sync.dma_start(out_d[384:512, cs], r2b[:, cs])

            if l0_early:
                emit_fb(0)
                emit_l0(0)
                emit_l0(1)
                if fb_splits == 3:
                    emit_fb(1, range(fb_grp))       # fresh buffer, no WAR
                    emit_l12(0)
                    emit_fb(1, range(fb_grp, F0))   # reuses fbh[0] after half-0
                else:
                    emit_l12(0)
                    emit_fb(1)
                if out_dma_split:
                    emit_out(0)
                emit_l12(1)
            else:
                for half in range(2):
                    emit_fb(half)
                    emit_l0(half)
                    emit_l12(half)
                    if out_dma_split:
                        emit_out(half)
            if out_dma_split:
                if l0_early:
                    emit_out(1)
            else:
                nc.sync.dma_start(out_d[0:128, :], r0)
                if n_layers >= 2:
                    nc.sync.dma_start(out_d[128:256, :], r1)
                if n_layers >= 3:
                    nc.sync.dma_start(out_d[256:384, :], r2a)
                    nc.sync.dma_start(out_d[384:512, :], r2b)

    nc.compile()
    return nc


def _host_prep(feat, W0, b0, W1, b1, W2, b2):
    """Rearrange full inputs into the per-core in_maps."""
    feat = np.ascontiguousarray(feat, dtype=np.float32)

    # W0: chunks c of 128 (i,j)-pairs, i-major: p = (i_local, j), i = 4c + p//32
    A = np.ascontiguousarray(W0.transpose(1, 2, 0)).reshape(F0 * F0, K)
    w0t = np.ascontiguousarray(
        A.reshape(NJ0, 128, K).transpose(1, 0, 2).reshape(128, NJ0 * K)
    ).astype(np.float16)
    w1t = np.ascontiguousarray(W1.transpose(1, 2, 0)).reshape(H, F0 * K).astype(np.float16)
    w2t = np.ascontiguousarray(W2.transpose(1, 2, 0)).reshape(H, F0 * K).astype(np.float16)

    p_ = np.arange(128)
    s4all = np.zeros((F0, NJ0 * 128), np.float16)
    for cc in range(NJ0):
        s4all[:, cc * 128 : (cc + 1) * 128] = (
            (4 * cc + p_[None, :] // F0) == np.arange(F0)[:, None]
        )

    b0t = np.ascontiguousarray(b0.reshape(2, 128).T).astype(np.float32)
    b1t = np.ascontiguousarray(b1.reshape(2, 128).T).astype(np.float32)
    b2t = np.ascontiguousarray(b2.reshape(2, 128).T).astype(np.float32)

    p = np.arange(128)
    in_maps = []
    for c in range(NCORES):
        fc = feat[c * BL : (c + 1) * BL]                        # [64, 32, 32]
        featT = np.ascontiguousarray(fc.transpose(1, 0, 2)).reshape(F0, BD)
        featT = featT.astype(np.float16)
        featR = np.ascontiguousarray(featT[p % F0])             # [128, BD]
        featH = np.concatenate(
            [
                featT[4 * cc + p // F0, t * NT : (t + 1) * NT]
                for t in range(T_TILES)
                for cc in range(NJ0)
            ],
            axis=1,
        )                                                        # [128, NJ0*BD] t-major
        in_maps.append(
            {
                "featT16": featT,
                "featR": featR,
                "featH": np.ascontiguousarray(featH),
                "s4all": s4all,
                "w0t": w0t,
                "w1t": w1t,
                "w2t": w2t,
                "b0t": b0t,
                "b1t": b1t,
                "b2t": b2t,
            }
        )
    return in_maps


def kernel(feat, W0, b0, W1, b1, W2, b2):
    global LAST_RESULTS
    if "nc" not in _CACHE:
        _CACHE["nc"] = _build_program()
    nc = _CACHE["nc"]
    in_maps = _host_prep(feat, W0, b0, W1, b1, W2, b2)
    res = run_bass_kernel_spmd(nc, in_maps, core_ids=list(range(NCORES)))
    LAST_RESULTS = res
    out = np.concatenate([res.results[c]["out"].T for c in range(NCORES)], axis=0)
    return np.ascontiguousarray(out, dtype=np.float32)
